# revision 43
# baseline (speedup 1.0000x reference)
"""Trainium2 Bass kernel for LocalSelfAttention (sliding-window, causal).

Problem: val (S=4096, B=2, D=768); q/k/v projections then Longformer-style
banded causal attention, window = 256 lookback (keys j in [i-256, i]).

Sharding: 8 cores = batch (2) x sequence quarters (4). Each core handles
1024 queries of one batch element and receives a 256-row key/value halo
(recomputed locally from val rows; no inter-core communication).

Math simplifications (exact up to float rounding):
  - bk dropped: per-query additive constant q.bk cancels in softmax.
  - bv added on host at the end: sum_j p_j (v0_j + bv) = (PV)/Z + bv.
  - no max-subtraction in softmax: scores ~ N(0,1), |s| < ~8, exp is safe.
  - 1/sqrt(hd) folded into Wq/bq on host.

Final structure (~130 us vs 210 us for the two-phase fp32r original):
  - All inputs bf16 (FWL weight loads at the PE 2.4 GHz roofline, half the
    DMA bytes). fp8 was measured out: elementwise errors on the q/k/v path
    transfer ~1:1 to the output (no sqrt-N averaging), ~2.8% >> the 2e-2
    gate. Masks ship as fp8 (0/1 exact).
  - DMA queues cluster completions near queue-end (engines drain
    descriptors round-robin), so assignment is the only lever: valT+wk on
    sync, bq/masks/wq on scalar, wv alone on the slow gpsimd queue.
  - Phase 1 needs no wv: all q/k projections + all 48 scores/exp/mask
    units, statically woven so psum rings (proj bufs=2, scores bufs=3)
    recycle under other PE work; all probs stay resident in SBUF (9 MB).
  - Phase 2: v projection woven with all PV units (B(m,g) gated on vaug
    tile availability), pv psum tiles share the proj ring slots.
  - Compact scores psum [128,768] per (head, 256-query group), col layout
    [kt1(256) kt0h(128) kt3h(128) kt2(256)] -- only the live halves of the
    outer key tiles are computed/exp'd/masked; the masked region is one
    contiguous 512-col DVE multiply (fp8 mask operand).
  - exp ACT table preloaded by a dummy activation at t~0 (saves ~2.7 us).
  - PV accumulates a head-pair into one psum bank [128, 260]:
    cols = qs*130 + hh*65 + (64 values + 1 ones-column row-sum); one wide
    DVE copy, two 128-row output DMAs alternating sync/scalar queues.
"""

import os
import numpy as np
import ml_dtypes

S, B, D = 4096, 2, 768
H, HD = 12, 64
W = 256
NCORES = 8
SQ = S // 4            # 1024 queries per core
SKV = SQ + W           # 1280 kv rows (halo)
NQT = SQ // 128        # 8 query tiles
NG = SQ // 256         # 4 query groups
ND = D // 128          # 6 feature tiles (also head pairs)
NKVT = SKV // 128      # 10 kv tiles
VA = HD + 1            # 65: per-head v width incl. ones column
SCALE = 1.0 / np.sqrt(HD).astype(np.float32)  # 0.125

_CACHE = {}


def _masks_np(boundary: bool) -> np.ndarray:
    """(2, 128, 768) fp8e4 multiplicative masks for the compact
    [kt1(256) kt0h(128) kt3h(128) kt2(256)] score layout (only the live
    halves of kt0/kt3 are computed). Partition = key-within-tile p; free =
    query row r within the group (c = r mod 128).

      kt1 block (cols 0:256):   r<128 all-valid; r>=128 tri c<=p
      kt0h block (cols 256:384): r<128 tri c<=p
      kt3h block (cols 384:512): r>=128 tri c>=p
      kt2 block (cols 512:768): r<128 tri c>=p; r>=128 all-valid
    Set 0 is used for group 0 (multiplied over the full 768 cols; on
    sequence-boundary cores kt1+kt0h are entirely invalid), set 1 for
    groups 1..3 (multiplied over cols 128:640 only). 0/1 are exact in fp8.
    """
    p = np.arange(128)[:, None]
    c = np.arange(128)[None, :]
    triL = (c <= p).astype(np.float32)
    triU = (c >= p).astype(np.float32)
    setB = np.ones((128, 768), np.float32)
    setB[:, 128:256] = triL
    setB[:, 256:384] = triL
    setB[:, 384:512] = triU
    setB[:, 512:640] = triU
    setA = setB.copy()
    if boundary:
        setA[:, 0:384] = 0.0    # kt1+kt0h keys are before row 0 -> invalid
    m = np.stack([setA, setB])
    return np.ascontiguousarray(m.astype(ml_dtypes.float8_e4m3fn))


def _build_nc():
    import concourse.bacc as bacc
    import concourse.mybir as mybir
    from concourse.tile import TileContext

    f32 = mybir.dt.float32
    bf16 = mybir.dt.bfloat16
    f8 = mybir.dt.float8e4
    AF = mybir.ActivationFunctionType

    nc = bacc.Bacc(trn_type="TRN2", debug=False, num_devices=NCORES)

    valT_d = nc.dram_tensor("valT", [D, SKV], bf16, kind="ExternalInput").ap()
    wq_d = nc.dram_tensor("wq", [D, D], bf16, kind="ExternalInput").ap()
    wk_d = nc.dram_tensor("wk", [D, D], bf16, kind="ExternalInput").ap()
    wv_d = nc.dram_tensor("wv", [D, D], bf16, kind="ExternalInput").ap()
    bq_d = nc.dram_tensor("bq", [D, 1], f32, kind="ExternalInput").ap()
    masks_d = nc.dram_tensor("masks", [2, 128, 768], f8, kind="ExternalInput").ap()
    out_d = nc.dram_tensor("out", [ND * NQT * 128, 2 * VA], f32, kind="ExternalOutput").ap()

    with TileContext(nc) as tc:
        with tc.tile_pool(name="persist", bufs=1) as pp, \
             tc.tile_pool(name="projps", bufs=2, space="PSUM") as projps, \
             tc.tile_pool(name="scps", bufs=3, space="PSUM") as scps, \
             tc.tile_pool(name="probsp", bufs=48) as prp, \
             tc.tile_pool(name="outp", bufs=6) as outp:

            qT = [pp.tile([128, SQ], bf16, name=f"qT{m}", tag=f"qT{m}") for m in range(ND)]
            kT = [pp.tile([128, SKV], bf16, name=f"kT{m}", tag=f"kT{m}") for m in range(ND)]
            vaug = [pp.tile([128, H * VA], bf16, name=f"vaug{t}", tag=f"vaug{t}") for t in range(NKVT)]
            bqt = [pp.tile([128, 1], f32, name=f"bqt{m}", tag=f"bqt{m}") for m in range(ND)]
            maskt = [pp.tile([128, 768], f8, name=f"maskt{i}", tag=f"maskt{i}") for i in range(2)]
            valT_t = [pp.tile([128, SKV], bf16, name=f"valTt{k}", tag=f"valTt{k}") for k in range(ND)]
            wq_t = [pp.tile([128, D], bf16, name=f"wqt{k}", tag=f"wqt{k}") for k in range(ND)]
            wk_t = [pp.tile([128, D], bf16, name=f"wkt{k}", tag=f"wkt{k}") for k in range(ND)]
            wv_t = [pp.tile([128, D], bf16, name=f"wvt{k}", tag=f"wvt{k}") for k in range(ND)]

            # ---- input DMAs. DMA engines drain each queue's descriptors
            # round-robin, so every DMA on a queue completes near that
            # queue's END: the lever is queue assignment, not order. The
            # whole scores pre-phase needs only valT+wq+wk; wv (only needed
            # by the final v/PV phase) rides the slow gpsimd software queue.
            for k in range(ND):
                nc.sync.dma_start(valT_t[k][:], valT_d[k * 128:(k + 1) * 128, :])
            for m in range(ND):
                nc.scalar.dma_start(bqt[m][:], bq_d[m * 128:(m + 1) * 128, :])
            for i in range(2):
                nc.scalar.dma_start(maskt[i][:], masks_d[i])
            for k in range(ND):
                nc.scalar.dma_start(wq_t[k][:], wq_d[k * 128:(k + 1) * 128, :])
            for k in range(ND):
                nc.scalar.dma_start(wk_t[k][:], wk_d[k * 128:(k + 1) * 128, :])
            # hold the wv issues back until wk has landed: wv isn't needed
            # until phase 2 (~70us) and otherwise steals early HBM bandwidth
            # from the start-gating valT/wq/wk transfers
            wvgate = pp.tile([128, 1], bf16, name="wvgate", tag="wvgate")
            nc.gpsimd.tensor_copy(wvgate[:], wk_t[ND - 1][:, 0:1])
            for k in range(ND):
                nc.gpsimd.dma_start(wv_t[k][:], wv_d[k * 128:(k + 1) * 128, :])
            for t in range(NKVT):
                ones_col = vaug[t][:].rearrange("p (h c) -> p h c", c=VA)[:, :, HD:VA]
                nc.vector.memset(ones_col, 1.0)

            # dummy exp at t~0 (no DMA deps) pulls the ~2.7us ACT table
            # load off the critical path of the first real softmax
            scratch = pp.tile([128, 1], f32, name="scratch", tag="scratch")
            nc.vector.memset(scratch[:], 0.0)
            nc.scalar.activation(scratch[:], scratch[:], AF.Exp)

            probs_ring = {}

            def unit_q(m, ch):
                ps = projps.tile([128, 512], f32, name="psq", tag="proj")
                for k in range(ND):
                    nc.tensor.matmul(
                        ps[:],
                        wq_t[k][:, m * 128:(m + 1) * 128],
                        valT_t[k][:, W + ch * 512:W + (ch + 1) * 512],
                        start=(k == 0), stop=(k == ND - 1))
                nc.scalar.activation(
                    qT[m][:, ch * 512:(ch + 1) * 512], ps[:],
                    AF.Identity, bias=bqt[m][:], scale=1.0)

            K_SPANS = ((0, 512), (512, 1024), (1024, 1280))

            def unit_k(m, j):
                lo, hi = K_SPANS[j]
                ps = projps.tile([128, hi - lo], f32, name="psk", tag="proj")
                for k in range(ND):
                    nc.tensor.matmul(
                        ps[:],
                        wk_t[k][:, m * 128:(m + 1) * 128],
                        valT_t[k][:, lo:hi],
                        start=(k == 0), stop=(k == ND - 1))
                nc.vector.tensor_copy(kT[m][:, lo:hi], ps[:])

            V_SPANS = ((0, 512, 0), (512, 768, 8))

            def unit_v(t, half):
                lo, hi, h0 = V_SPANS[half]
                # phase 2 only: share the idle 3-deep scores ring (see unit_b)
                ps = scps.tile([128, hi - lo], f32, name="psv", tag="scores")
                for k in range(ND):
                    nc.tensor.matmul(
                        ps[:],
                        valT_t[k][:, t * 128:(t + 1) * 128],
                        wv_t[k][:, lo:hi],
                        start=(k == 0), stop=(k == ND - 1))
                nh = (hi - lo) // HD
                src = ps[:].rearrange("p (h c) -> p h c", c=HD)
                dst = vaug[t][:].rearrange("p (h c) -> p h c", c=VA)[:, h0:h0 + nh, 0:HD]
                nc.vector.tensor_copy(dst, src)

            # compact scores layout [kt1(256) kt0h(128) kt3h(128) kt2(256)]:
            # (col, ka_offset, q_lo, q_width) -- only live halves of kt0/kt3
            SC_BLOCKS = ((0, 1, 0, 256), (256, 0, 0, 128),
                         (384, 3, 128, 128), (512, 2, 0, 256))

            def unit_a(m, hh, g):
                ph = hh * 64
                ps = scps.tile([128, 768], f32, name="pss", tag="scores")
                for col, kto, qlo, qw in SC_BLOCKS:
                    ka = 2 * g + kto
                    q0 = g * 256 + qlo
                    nc.tensor.matmul(
                        ps[:, col:col + qw],
                        kT[m][ph:ph + 64, ka * 128:(ka + 1) * 128],
                        qT[m][ph:ph + 64, q0:q0 + qw],
                        start=True, stop=True)
                probs = prp.tile([128, 768], bf16, name="probs", tag="probs")
                nc.scalar.activation(probs[:], ps[:], AF.Exp)
                if g == 0:
                    nc.vector.tensor_mul(probs[:], probs[:], maskt[0][:])
                else:
                    nc.vector.tensor_mul(
                        probs[:, 128:640], probs[:, 128:640], maskt[1][:, 128:640])
                probs_ring[(m, hh, g)] = probs

            # probs col ranges per query half qs: (ka_offset, col)
            PV_SLICES = (((1, 0), (0, 256), (2, 512)),      # qs=0: kt1,kt0h,kt2 @ r<128
                         ((1, 128), (3, 384), (2, 640)))    # qs=1: kt1,kt3h,kt2 @ r>=128

            def unit_b(m, g):
                # phase 2 only: the scores ring (3 x 2-bank slots) is idle
                # once all A units are done -- deeper ring than projps
                pv = scps.tile([128, 2 * 2 * VA], f32, name="pspv", tag="scores")
                for qs in range(2):
                    for hh in range(2):
                        h = 2 * m + hh
                        probs = probs_ring[(m, hh, g)]
                        co = qs * 2 * VA + hh * VA
                        for n, (kto, c) in enumerate(PV_SLICES[qs]):
                            nc.tensor.matmul(
                                pv[:, co:co + VA],
                                probs[:, c:c + 128],
                                vaug[2 * g + kto][:, h * VA:(h + 1) * VA],
                                start=(n == 0), stop=(n == 2))
                osb = outp.tile([128, 4 * VA], f32, name="osb", tag="outsb")
                nc.vector.tensor_copy(osb[:], pv[:])
                eng = nc.sync if (m + g) % 2 == 0 else nc.scalar
                for qs in range(2):
                    row = (m * NQT + 2 * g + qs) * 128
                    eng.dma_start(out_d[row:row + 128, :],
                                  osb[:, qs * 2 * VA:(qs + 1) * 2 * VA])
                del probs_ring[(m, 0, g)]
                del probs_ring[(m, 1, g)]

            # ---------------- static emission weave ----------------
            def emit(u):
                if u[0] == "A":
                    unit_a(*u[1])
                elif u[0] == "B":
                    unit_b(*u[1])
                elif u[0] == "V":
                    unit_v(*u[1])
                else:
                    kind, mm, j = u[1]
                    if kind == "q":
                        unit_q(mm, j)
                    else:
                        unit_k(mm, j)

            def q_units(m):
                return [("P", ("q", m, 0)), ("P", ("q", m, 1))]

            def k_units(m):
                return [("P", ("k", m, 0)), ("P", ("k", m, 1)), ("P", ("k", m, 2))]

            def a_units(m):
                return [("A", (m, hh, g)) for g in range(NG) for hh in range(2)]

            # Phase 1 (input DMAs landing): all q/k projections and all
            # scores/exp/mask units -- nothing here touches wv. q units run
            # first (wq rides the small fast scalar queue; wk lands with
            # valT on sync). probs for every (m,h,g) stay resident in SBUF.
            for u in q_units(0) + q_units(1) + k_units(0):
                emit(u)
            for m in range(ND):
                p_next = (q_units(m + 2) if m < ND - 2 else []) + \
                         (k_units(m + 1) if m < ND - 1 else [])
                for idx, u in enumerate(a_units(m)):
                    emit(u)
                    if idx % 2 == 1 and p_next:
                        emit(p_next.pop(0))
                while p_next:
                    emit(p_next.pop(0))

            # Phase 2: v projection woven with all PV units. B(m,g) needs
            # vaug tiles <= 2g+3, i.e. v units 0..4g+7; emit B g-major so
            # gating clears as v tiles appear.
            vseq = [("V", (t, half)) for t in range(NKVT) for half in range(2)]
            bpend = [(m, g) for g in range(NG) for m in range(ND)]
            vi = 0
            while vseq or bpend:
                if bpend and vi >= 4 * bpend[0][1] + 8:
                    emit(("B", bpend.pop(0)))
                if vseq:
                    emit(vseq.pop(0)); vi += 1
                # once the g<=2 gates have cleared, drain the remaining v
                # units so their vaug casts hit DVE before the tail B
                # units' out-copies (which would otherwise delay them)
                if vi == 16:
                    while vseq:
                        emit(vseq.pop(0)); vi += 1

    nc.compile()
    return nc


def _get_nc():
    if "nc" not in _CACHE:
        _CACHE["nc"] = _build_nc()
    return _CACHE["nc"]


def _install_ntff_hook():
    """Provide antenv.axon_hooks (absent in this image) so bass_utils can
    NTFF-profile under axon, using trn_agent_boot's ctypes hook builder."""
    import sys
    import types
    try:
        from antenv.axon_hooks import get_axon_ntff_profile_hook  # noqa: F401
        return
    except ImportError:
        pass
    try:
        import antenv
        from trn_agent_boot.trn_boot import _ntff_profile_via_ctypes
        hook = _ntff_profile_via_ctypes("/opt/axon/libaxon_pjrt.so")
        mod = types.ModuleType("antenv.axon_hooks")
        mod.get_axon_ntff_profile_hook = lambda: hook
        mod.set_axon_ntff_profile_hook = lambda h: None
        sys.modules["antenv.axon_hooks"] = mod
        antenv.axon_hooks = mod
    except Exception as e:  # profiling is best-effort
        print(f"ntff hook install failed: {e}")


def kernel(val, Wq, bq, Wk, bk, Wv, bv):
    from concourse.bass_utils import run_bass_kernel_spmd

    bf = ml_dtypes.bfloat16
    val = np.asarray(val, dtype=np.float32)
    Wq = np.asarray(Wq, dtype=np.float32)
    bq = np.asarray(bq, dtype=np.float32)
    Wk = np.asarray(Wk, dtype=np.float32)
    Wv = np.asarray(Wv, dtype=np.float32)
    bv = np.asarray(bv, dtype=np.float32)

    wq_s = np.ascontiguousarray((Wq * SCALE).astype(bf))
    bq_s = np.ascontiguousarray((bq * SCALE).reshape(D, 1))
    wk_c = np.ascontiguousarray(Wk.astype(bf))
    wv_c = np.ascontiguousarray(Wv.astype(bf))

    in_maps = []
    for c in range(NCORES):
        b, qd = divmod(c, 4)
        lo = qd * SQ - W
        hi = qd * SQ + SQ
        vs = val[max(lo, 0):hi, b, :]
        if lo < 0:
            vs = np.concatenate([np.zeros((-lo, D), np.float32), vs], axis=0)
        in_maps.append({
            "valT": np.ascontiguousarray(vs.T.astype(bf)),
            "wq": wq_s, "wk": wk_c, "wv": wv_c, "bq": bq_s,
            "masks": _masks_np(boundary=(qd == 0)),
        })

    nc = _get_nc()
    trace = os.environ.get("BASS_KERNEL_TRACE", "0") == "1"
    kwargs = {}
    if trace:
        _install_ntff_hook()
        kwargs = dict(trace=True, tmpdir=os.environ.get("BASS_KERNEL_TRACE_DIR") or None)
    res = run_bass_kernel_spmd(nc, in_maps, list(range(NCORES)), **kwargs)
    _CACHE["last_result"] = res

    out = np.empty((S, B, D), np.float32)
    for c in range(NCORES):
        b, qd = divmod(c, 4)
        raw = res.results[c]["out"].reshape(ND, NQT, 128, 2, VA)
        a = raw.transpose(1, 2, 0, 3, 4).reshape(SQ, H, VA)
        out[qd * SQ:(qd + 1) * SQ, b, :] = (
            a[:, :, 0:HD] / a[:, :, HD:VA]).reshape(SQ, D)
    out += bv
    return out


# revision 46
# speedup vs baseline: 1.0209x; 1.0209x over previous
"""Trainium2 Bass kernel for LocalSelfAttention (sliding-window, causal).

Problem: val (S=4096, B=2, D=768); q/k/v projections then Longformer-style
banded causal attention, window = 256 lookback (keys j in [i-256, i]).

Sharding: 8 cores = batch (2) x sequence quarters (4). Each core handles
1024 queries of one batch element and receives a 256-row key/value halo
(recomputed locally from val rows; no inter-core communication).

Math simplifications (exact up to float rounding):
  - bk dropped: per-query additive constant q.bk cancels in softmax.
  - bv added on host at the end: sum_j p_j (v0_j + bv) = (PV)/Z + bv.
  - no max-subtraction in softmax: scores ~ N(0,1), |s| < ~8, exp is safe.
  - 1/sqrt(hd) folded into Wq/bq on host.

Final structure (~130 us vs 210 us for the two-phase fp32r original):
  - All inputs bf16 (FWL weight loads at the PE 2.4 GHz roofline, half the
    DMA bytes). fp8 was measured out: elementwise errors on the q/k/v path
    transfer ~1:1 to the output (no sqrt-N averaging), ~2.8% >> the 2e-2
    gate. Masks ship as fp8 (0/1 exact).
  - DMA queues cluster completions near queue-end (engines drain
    descriptors round-robin), so assignment is the only lever: valT+wk on
    sync, bq/masks/wq on scalar, wv alone on the slow gpsimd queue.
  - Phase 1 needs no wv: all q/k projections + all 48 scores/exp/mask
    units, statically woven so psum rings (proj bufs=2, scores bufs=3)
    recycle under other PE work; all probs stay resident in SBUF (9 MB).
  - Phase 2: v projection woven with all PV units (B(m,g) gated on vaug
    tile availability), pv psum tiles share the proj ring slots.
  - Compact scores psum [128,768] per (head, 256-query group), col layout
    [kt1(256) kt0h(128) kt3h(128) kt2(256)] -- only the live halves of the
    outer key tiles are computed/exp'd/masked; the masked region is one
    contiguous 512-col DVE multiply (fp8 mask operand).
  - exp ACT table preloaded by a dummy activation at t~0 (saves ~2.7 us).
  - PV accumulates a head-pair into one psum bank [128, 260]:
    cols = qs*130 + hh*65 + (64 values + 1 ones-column row-sum); one wide
    DVE copy, two 128-row output DMAs alternating sync/scalar queues.
"""

import os
import numpy as np
import ml_dtypes

S, B, D = 4096, 2, 768
H, HD = 12, 64
W = 256
NCORES = 8
SQ = S // 4            # 1024 queries per core
SKV = SQ + W           # 1280 kv rows (halo)
NQT = SQ // 128        # 8 query tiles
NG = SQ // 256         # 4 query groups
ND = D // 128          # 6 feature tiles (also head pairs)
NKVT = SKV // 128      # 10 kv tiles
VA = HD + 1            # 65: per-head v width incl. ones column
SCALE = 1.0 / np.sqrt(HD).astype(np.float32)  # 0.125

_CACHE = {}


def _masks_np(boundary: bool) -> np.ndarray:
    """(2, 128, 768) fp8e4 multiplicative masks for the compact
    [kt1(256) kt0h(128) kt3h(128) kt2(256)] score layout (only the live
    halves of kt0/kt3 are computed). Partition = key-within-tile p; free =
    query row r within the group (c = r mod 128).

      kt1 block (cols 0:256):   r<128 all-valid; r>=128 tri c<=p
      kt0h block (cols 256:384): r<128 tri c<=p
      kt3h block (cols 384:512): r>=128 tri c>=p
      kt2 block (cols 512:768): r<128 tri c>=p; r>=128 all-valid
    Set 0 is used for group 0 (multiplied over the full 768 cols; on
    sequence-boundary cores kt1+kt0h are entirely invalid), set 1 for
    groups 1..3 (multiplied over cols 128:640 only). 0/1 are exact in fp8.
    """
    p = np.arange(128)[:, None]
    c = np.arange(128)[None, :]
    triL = (c <= p).astype(np.float32)
    triU = (c >= p).astype(np.float32)
    setB = np.ones((128, 768), np.float32)
    setB[:, 128:256] = triL
    setB[:, 256:384] = triL
    setB[:, 384:512] = triU
    setB[:, 512:640] = triU
    setA = setB.copy()
    if boundary:
        setA[:, 0:384] = 0.0    # kt1+kt0h keys are before row 0 -> invalid
    m = np.stack([setA, setB])
    return np.ascontiguousarray(m.astype(ml_dtypes.float8_e4m3fn))


def _build_nc():
    import concourse.bacc as bacc
    import concourse.mybir as mybir
    from concourse.tile import TileContext

    f32 = mybir.dt.float32
    bf16 = mybir.dt.bfloat16
    f8 = mybir.dt.float8e4
    AF = mybir.ActivationFunctionType

    nc = bacc.Bacc(trn_type="TRN2", debug=False, num_devices=NCORES)

    valT_d = nc.dram_tensor("valT", [D, SKV], bf16, kind="ExternalInput").ap()
    wq_d = nc.dram_tensor("wq", [D, D], bf16, kind="ExternalInput").ap()
    wk_d = nc.dram_tensor("wk", [D, D], bf16, kind="ExternalInput").ap()
    wv_d = nc.dram_tensor("wv", [D, D], bf16, kind="ExternalInput").ap()
    bq_d = nc.dram_tensor("bq", [D, 1], f32, kind="ExternalInput").ap()
    masks_d = nc.dram_tensor("masks", [2, 128, 768], f8, kind="ExternalInput").ap()
    out_d = nc.dram_tensor("out", [ND * NQT * 128, 2 * VA], f32, kind="ExternalOutput").ap()

    with TileContext(nc) as tc:
        with tc.tile_pool(name="persist", bufs=1) as pp, \
             tc.tile_pool(name="projps", bufs=2, space="PSUM") as projps, \
             tc.tile_pool(name="scps", bufs=3, space="PSUM") as scps, \
             tc.tile_pool(name="probsp", bufs=48) as prp, \
             tc.tile_pool(name="outp", bufs=6) as outp:

            qT = [pp.tile([128, SQ], bf16, name=f"qT{m}", tag=f"qT{m}") for m in range(ND)]
            kT = [pp.tile([128, SKV], bf16, name=f"kT{m}", tag=f"kT{m}") for m in range(ND)]
            vaug = [pp.tile([128, H * VA], bf16, name=f"vaug{t}", tag=f"vaug{t}") for t in range(NKVT)]
            bqt = [pp.tile([128, 1], f32, name=f"bqt{m}", tag=f"bqt{m}") for m in range(ND)]
            maskt = [pp.tile([128, 768], f8, name=f"maskt{i}", tag=f"maskt{i}") for i in range(2)]
            valT_t = [pp.tile([128, SKV], bf16, name=f"valTt{k}", tag=f"valTt{k}") for k in range(ND)]
            wq_t = [pp.tile([128, D], bf16, name=f"wqt{k}", tag=f"wqt{k}") for k in range(ND)]
            wk_t = [pp.tile([128, D], bf16, name=f"wkt{k}", tag=f"wkt{k}") for k in range(ND)]
            wv_t = [pp.tile([128, D], bf16, name=f"wvt{k}", tag=f"wvt{k}") for k in range(ND)]

            # ---- input DMAs. DMA engines drain each queue's descriptors
            # round-robin, so every DMA on a queue completes near that
            # queue's END: the lever is queue assignment, not order. The
            # whole scores pre-phase needs only valT+wq+wk; wv (only needed
            # by the final v/PV phase) rides the slow gpsimd software queue.
            for k in range(ND):
                nc.sync.dma_start(valT_t[k][:], valT_d[k * 128:(k + 1) * 128, :])
            for m in range(ND):
                nc.scalar.dma_start(bqt[m][:], bq_d[m * 128:(m + 1) * 128, :])
            for i in range(2):
                nc.scalar.dma_start(maskt[i][:], masks_d[i])
            for k in range(ND):
                nc.scalar.dma_start(wq_t[k][:], wq_d[k * 128:(k + 1) * 128, :])
            for k in range(ND):
                nc.scalar.dma_start(wk_t[k][:], wk_d[k * 128:(k + 1) * 128, :])
            for k in range(ND):
                nc.gpsimd.dma_start(wv_t[k][:], wv_d[k * 128:(k + 1) * 128, :])
            for t in range(NKVT):
                ones_col = vaug[t][:].rearrange("p (h c) -> p h c", c=VA)[:, :, HD:VA]
                nc.vector.memset(ones_col, 1.0)

            # dummy exp at t~0 (no DMA deps) pulls the ~2.7us ACT table
            # load off the critical path of the first real softmax
            scratch = pp.tile([128, 1], f32, name="scratch", tag="scratch")
            nc.vector.memset(scratch[:], 0.0)
            nc.scalar.activation(scratch[:], scratch[:], AF.Exp)

            probs_ring = {}

            def unit_q(m, ch):
                ps = projps.tile([128, 512], f32, name="psq", tag="proj")
                for k in range(ND):
                    nc.tensor.matmul(
                        ps[:],
                        wq_t[k][:, m * 128:(m + 1) * 128],
                        valT_t[k][:, W + ch * 512:W + (ch + 1) * 512],
                        start=(k == 0), stop=(k == ND - 1))
                nc.scalar.activation(
                    qT[m][:, ch * 512:(ch + 1) * 512], ps[:],
                    AF.Identity, bias=bqt[m][:], scale=1.0)

            K_SPANS = ((0, 512), (512, 1024), (1024, 1280))

            def unit_k(m, j):
                lo, hi = K_SPANS[j]
                ps = projps.tile([128, hi - lo], f32, name="psk", tag="proj")
                for k in range(ND):
                    nc.tensor.matmul(
                        ps[:],
                        wk_t[k][:, m * 128:(m + 1) * 128],
                        valT_t[k][:, lo:hi],
                        start=(k == 0), stop=(k == ND - 1))
                nc.vector.tensor_copy(kT[m][:, lo:hi], ps[:])

            V_SPANS = ((0, 512, 0), (512, 768, 8))

            def unit_v(t, half):
                lo, hi, h0 = V_SPANS[half]
                ps = projps.tile([128, hi - lo], f32, name="psv", tag="proj")
                for k in range(ND):
                    nc.tensor.matmul(
                        ps[:],
                        valT_t[k][:, t * 128:(t + 1) * 128],
                        wv_t[k][:, lo:hi],
                        start=(k == 0), stop=(k == ND - 1))
                nh = (hi - lo) // HD
                src = ps[:].rearrange("p (h c) -> p h c", c=HD)
                dst = vaug[t][:].rearrange("p (h c) -> p h c", c=VA)[:, h0:h0 + nh, 0:HD]
                nc.vector.tensor_copy(dst, src)

            # compact scores layout [kt1(256) kt0h(128) kt3h(128) kt2(256)]:
            # (col, ka_offset, q_lo, q_width) -- only live halves of kt0/kt3
            SC_BLOCKS = ((0, 1, 0, 256), (256, 0, 0, 128),
                         (384, 3, 128, 128), (512, 2, 0, 256))

            def unit_a(m, hh, g):
                ph = hh * 64
                ps = scps.tile([128, 768], f32, name="pss", tag="scores")
                for col, kto, qlo, qw in SC_BLOCKS:
                    ka = 2 * g + kto
                    q0 = g * 256 + qlo
                    nc.tensor.matmul(
                        ps[:, col:col + qw],
                        kT[m][ph:ph + 64, ka * 128:(ka + 1) * 128],
                        qT[m][ph:ph + 64, q0:q0 + qw],
                        start=True, stop=True)
                probs = prp.tile([128, 768], bf16, name="probs", tag="probs")
                nc.scalar.activation(probs[:], ps[:], AF.Exp)
                if g == 0:
                    nc.vector.tensor_mul(probs[:], probs[:], maskt[0][:])
                else:
                    nc.vector.tensor_mul(
                        probs[:, 128:640], probs[:, 128:640], maskt[1][:, 128:640])
                probs_ring[(m, hh, g)] = probs

            # probs col ranges per query half qs: (ka_offset, col)
            PV_SLICES = (((1, 0), (0, 256), (2, 512)),      # qs=0: kt1,kt0h,kt2 @ r<128
                         ((1, 128), (3, 384), (2, 640)))    # qs=1: kt1,kt3h,kt2 @ r>=128

            def unit_b(m, g):
                # phase 2 only: the scores ring (3 x 2-bank slots) is idle
                # once all A units are done -- deeper ring than projps
                pv = scps.tile([128, 2 * 2 * VA], f32, name="pspv", tag="scores")
                for qs in range(2):
                    for hh in range(2):
                        h = 2 * m + hh
                        probs = probs_ring[(m, hh, g)]
                        co = qs * 2 * VA + hh * VA
                        for n, (kto, c) in enumerate(PV_SLICES[qs]):
                            nc.tensor.matmul(
                                pv[:, co:co + VA],
                                probs[:, c:c + 128],
                                vaug[2 * g + kto][:, h * VA:(h + 1) * VA],
                                start=(n == 0), stop=(n == 2))
                osb = outp.tile([128, 4 * VA], f32, name="osb", tag="outsb")
                nc.vector.tensor_copy(osb[:], pv[:])
                eng = nc.sync if (m + g) % 2 == 0 else nc.scalar
                for qs in range(2):
                    row = (m * NQT + 2 * g + qs) * 128
                    eng.dma_start(out_d[row:row + 128, :],
                                  osb[:, qs * 2 * VA:(qs + 1) * 2 * VA])
                del probs_ring[(m, 0, g)]
                del probs_ring[(m, 1, g)]

            # ---------------- static emission weave ----------------
            def emit(u):
                if u[0] == "A":
                    unit_a(*u[1])
                elif u[0] == "B":
                    unit_b(*u[1])
                elif u[0] == "V":
                    unit_v(*u[1])
                else:
                    kind, mm, j = u[1]
                    if kind == "q":
                        unit_q(mm, j)
                    else:
                        unit_k(mm, j)

            def q_units(m):
                return [("P", ("q", m, 0)), ("P", ("q", m, 1))]

            def k_units(m):
                return [("P", ("k", m, 0)), ("P", ("k", m, 1)), ("P", ("k", m, 2))]

            def a_units(m):
                return [("A", (m, hh, g)) for g in range(NG) for hh in range(2)]

            # Phase 1 (input DMAs landing): all q/k projections and all
            # scores/exp/mask units -- nothing here touches wv. q units run
            # first (wq rides the small fast scalar queue; wk lands with
            # valT on sync). probs for every (m,h,g) stay resident in SBUF.
            for u in q_units(0) + q_units(1) + k_units(0):
                emit(u)
            for m in range(ND):
                p_next = (q_units(m + 2) if m < ND - 2 else []) + \
                         (k_units(m + 1) if m < ND - 1 else [])
                for idx, u in enumerate(a_units(m)):
                    emit(u)
                    if idx % 2 == 1 and p_next:
                        emit(p_next.pop(0))
                while p_next:
                    emit(p_next.pop(0))

            # Phase 2: v projection woven with all PV units. B(m,g) needs
            # vaug tiles <= 2g+3, i.e. v units 0..4g+7; emit B g-major so
            # gating clears as v tiles appear.
            vseq = [("V", (t, half)) for t in range(NKVT) for half in range(2)]
            bpend = [(m, g) for g in range(NG) for m in range(ND)]
            vi = 0
            while vseq or bpend:
                if bpend and vi >= 4 * bpend[0][1] + 8:
                    emit(("B", bpend.pop(0)))
                if vseq:
                    emit(vseq.pop(0)); vi += 1

    nc.compile()
    return nc


def _get_nc():
    if "nc" not in _CACHE:
        _CACHE["nc"] = _build_nc()
    return _CACHE["nc"]


def _install_ntff_hook():
    """Provide antenv.axon_hooks (absent in this image) so bass_utils can
    NTFF-profile under axon, using trn_agent_boot's ctypes hook builder."""
    import sys
    import types
    try:
        from antenv.axon_hooks import get_axon_ntff_profile_hook  # noqa: F401
        return
    except ImportError:
        pass
    try:
        import antenv
        from trn_agent_boot.trn_boot import _ntff_profile_via_ctypes
        hook = _ntff_profile_via_ctypes("/opt/axon/libaxon_pjrt.so")
        mod = types.ModuleType("antenv.axon_hooks")
        mod.get_axon_ntff_profile_hook = lambda: hook
        mod.set_axon_ntff_profile_hook = lambda h: None
        sys.modules["antenv.axon_hooks"] = mod
        antenv.axon_hooks = mod
    except Exception as e:  # profiling is best-effort
        print(f"ntff hook install failed: {e}")


def kernel(val, Wq, bq, Wk, bk, Wv, bv):
    from concourse.bass_utils import run_bass_kernel_spmd

    bf = ml_dtypes.bfloat16
    val = np.asarray(val, dtype=np.float32)
    Wq = np.asarray(Wq, dtype=np.float32)
    bq = np.asarray(bq, dtype=np.float32)
    Wk = np.asarray(Wk, dtype=np.float32)
    Wv = np.asarray(Wv, dtype=np.float32)
    bv = np.asarray(bv, dtype=np.float32)

    wq_s = np.ascontiguousarray((Wq * SCALE).astype(bf))
    bq_s = np.ascontiguousarray((bq * SCALE).reshape(D, 1))
    wk_c = np.ascontiguousarray(Wk.astype(bf))
    wv_c = np.ascontiguousarray(Wv.astype(bf))

    in_maps = []
    for c in range(NCORES):
        b, qd = divmod(c, 4)
        lo = qd * SQ - W
        hi = qd * SQ + SQ
        vs = val[max(lo, 0):hi, b, :]
        if lo < 0:
            vs = np.concatenate([np.zeros((-lo, D), np.float32), vs], axis=0)
        in_maps.append({
            "valT": np.ascontiguousarray(vs.T.astype(bf)),
            "wq": wq_s, "wk": wk_c, "wv": wv_c, "bq": bq_s,
            "masks": _masks_np(boundary=(qd == 0)),
        })

    nc = _get_nc()
    trace = os.environ.get("BASS_KERNEL_TRACE", "0") == "1"
    kwargs = {}
    if trace:
        _install_ntff_hook()
        kwargs = dict(trace=True, tmpdir=os.environ.get("BASS_KERNEL_TRACE_DIR") or None)
    res = run_bass_kernel_spmd(nc, in_maps, list(range(NCORES)), **kwargs)
    _CACHE["last_result"] = res

    out = np.empty((S, B, D), np.float32)
    for c in range(NCORES):
        b, qd = divmod(c, 4)
        raw = res.results[c]["out"].reshape(ND, NQT, 128, 2, VA)
        a = raw.transpose(1, 2, 0, 3, 4).reshape(SQ, H, VA)
        out[qd * SQ:(qd + 1) * SQ, b, :] = (
            a[:, :, 0:HD] / a[:, :, HD:VA]).reshape(SQ, D)
    out += bv
    return out
